# revision 25
# baseline (speedup 1.0000x reference)
"""Trainium2 Bass kernel for nn_BDH_GPU (sparse linear-attention decoder).

Self-contained: builds an SPMD Bass/Tile program for 8 NeuronCores,
shards batch(2) x head-groups(4), runs via PJRT (axon), gathers output.

Sharding: core c -> batch b=c//4, heads [4*(c%4), 4*(c%4)+4).
Per-layer AllReduce of y@encoder partial [D,T] within each 4-core group.

Layouts: activations transposed [feature(part), token(free)].
Host folds: RoPE de-interleave permutation into decoder_x/decoder_y cols
and encoder rows; mean_d(v)==0 exploited (v rows are LN outputs);
LN(a) scale folded into the PSUM->SBUF copy of a.
"""
import numpy as np
import ml_dtypes

import concourse.bass as bass
import concourse.tile as tile
import concourse.mybir as mybir
from concourse import bacc, bass2jax
from concourse.masks import make_identity

AF = mybir.ActivationFunctionType
FP32 = mybir.dt.float32
BF16 = mybir.dt.bfloat16
ts = bass.ts

D, H, N, VOCAB, L, SD, B, T = 1024, 16, 8192, 32000, 4, 512, 2, 1024
NCORES = 8
NHC = 4           # heads per core
VSH = VOCAB // 4  # vocab shard per core (within batch group) = 8000
VCH = 500         # vocab N-chunk (<=512 f32 psum bank)
NVC = VSH // VCH  # 16
EPS = 1e-5

_CACHE = {}


def build_program(nlayers=L, repeat=1, do_readout=True, collective=True,
                  af_act=False, p5b=3, p2b=3, apb=2):
    nc = bacc.Bacc("TRN2", target_bir_lowering=False, debug=False,
                   num_devices=NCORES)
    CDT = BF16

    v0T_f = nc.dram_tensor("v0t_f", [D, T], FP32, kind="ExternalInput")
    v0T_c = nc.dram_tensor("v0t_c", [D, T], CDT, kind="ExternalInput")
    v0n_c = nc.dram_tensor("v0n_c", [T, D], CDT, kind="ExternalInput")
    wx_d = nc.dram_tensor("wx", [NHC, D, SD], CDT, kind="ExternalInput")
    wy_d = nc.dram_tensor("wy", [NHC, D, SD], CDT, kind="ExternalInput")
    enc_d = nc.dram_tensor("enc", [NHC * SD, D], CDT, kind="ExternalInput")
    ro_d = nc.dram_tensor("ro", [D, VSH], CDT, kind="ExternalInput")
    cos_d = nc.dram_tensor("cos", [SD // 2, T], CDT, kind="ExternalInput")
    sin_d = nc.dram_tensor("sin", [SD // 2, T], CDT, kind="ExternalInput")
    msk_d = nc.dram_tensor("msk", [2, 128, 256], CDT, kind="ExternalInput")
    out_d = nc.dram_tensor("logits", [T, VSH], FP32, kind="ExternalOutput")

    with tile.TileContext(nc) as tc:
        with (
            tc.tile_pool(name="res", bufs=1) as res,
            tc.tile_pool(name="act", bufs=1) as act,
            tc.tile_pool(name="wst", bufs=10) as wst,
            tc.tile_pool(name="est", bufs=6) as est,
            tc.tile_pool(name="sml", bufs=2) as sml,
            tc.tile_pool(name="stg", bufs=4) as stg,
            tc.tile_pool(name="psp", bufs=3, space="PSUM") as psp,
            tc.tile_pool(name="dram", bufs=2, space="DRAM") as dram,
        ):
            def P5(nm):
                return psp.tile([128, 512], FP32, tag="p5", bufs=p5b, name=nm)

            def P2(nm):
                return psp.tile([128, 256], FP32, tag="p2", bufs=p2b, name=nm)

            def PA(nm):
                return psp.tile([128, 256], FP32, tag="ap2", bufs=apb, name=nm)

            # ---- constants ----
            cosv, sinv, masks = [], [], []
            for i in range(2):
                ct = res.tile([128, T], CDT, name=f"cos{i}")
                nc.sync.dma_start(ct[:], cos_d[ts(i, 128), :])
                cosv.append(ct)
                st = res.tile([128, T], CDT, name=f"sin{i}")
                nc.sync.dma_start(st[:], sin_d[ts(i, 128), :])
                sinv.append(st)
            for i in range(2):
                mt = res.tile([128, 256], CDT, name=f"msk{i}")
                nc.sync.dma_start(mt[:], msk_d[i])
                masks.append(mt)
            ones = res.tile([128, 128], CDT, name="ones")
            nc.vector.memset(ones[:], 1.0)
            epst = res.tile([128, 1], FP32, name="epst")
            nc.vector.memset(epst[:], EPS)

            def load_v(sfx):
                vT_f, vT_c, vn_c = [], [], []
                for k in range(8):
                    a = res.tile([128, T], FP32, tag=f"vTf{k}", name=f"vTf{k}_{sfx}")
                    nc.sync.dma_start(a[:], v0T_f[ts(k, 128), :])
                    vT_f.append(a)
                    b = res.tile([128, T], CDT, tag=f"vTc{k}", name=f"vTc{k}_{sfx}")
                    nc.sync.dma_start(b[:], v0T_c[ts(k, 128), :])
                    vT_c.append(b)
                    c = res.tile([128, D], CDT, tag=f"vnc{k}", name=f"vnc{k}_{sfx}")
                    nc.sync.dma_start(c[:], v0n_c[ts(k, 128), :])
                    vn_c.append(c)
                return vT_f, vT_c, vn_c

            vT_f, vT_c, vn_c = load_v("init")

            for rep in range(repeat):
                if rep > 0:
                    vT_f, vT_c, vn_c = load_v(f"r{rep}")

                for layer in range(nlayers):
                    tg = f"r{rep}l{layer}"
                    ar_in = dram.tile([D, T], FP32, tag="ar_in", name=f"ari_{tg}")
                    ar_out = dram.tile([D, T], FP32, tag="ar_out", name=f"aro_{tg}")

                    ytiles = {}
                    for h in range(NHC):
                        hg = f"{tg}h{h}"
                        # ---- x'T = relu(Wx_h^T @ vT) ----
                        wxt = []
                        for k in range(8):
                            w = wst.tile([128, SD], CDT, tag="wtile",
                                         name=f"wx_{hg}k{k}")
                            nc.sync.dma_start(w[:], wx_d[h, ts(k, 128), :])
                            wxt.append(w)
                        xp = []
                        for m in range(4):
                            x = act.tile([128, T], CDT, tag=f"xp{m}", bufs=1,
                                         name=f"xp{m}_{hg}")
                            xp.append(x)
                            for jj in range(2):
                                ps = P5(f"xps_{hg}m{m}j{jj}")
                                for k in range(8):
                                    nc.tensor.matmul(
                                        ps[:], wxt[k][:, ts(m, 128)],
                                        vT_c[k][:, ts(jj, 512)],
                                        start=(k == 0), stop=(k == 7))
                                nc.scalar.activation(
                                    out=x[:, ts(jj, 512)], in_=ps[:], func=AF.Relu)
                        # ---- rope: qr (= kr); de-interleave folded on host ----
                        qr = [act.tile([128, T], CDT, tag=f"qr{i}",
                                       name=f"qr{i}_{hg}") for i in range(4)]
                        for i in range(2):
                            t1 = sml.tile([128, T], CDT, tag="ropet1", bufs=2,
                                          name=f"t1_{hg}i{i}")
                            nc.gpsimd.tensor_mul(t1[:], xp[i][:], cosv[i][:])
                            nc.gpsimd.tensor_mul(qr[i][:], xp[2 + i][:], sinv[i][:])
                            nc.vector.tensor_sub(qr[i][:], t1[:], qr[i][:])
                            t3 = sml.tile([128, T], CDT, tag="ropet1", bufs=2,
                                          name=f"t3_{hg}i{i}")
                            nc.gpsimd.tensor_mul(t3[:], xp[i][:], sinv[i][:])
                            nc.gpsimd.tensor_mul(qr[2 + i][:], xp[2 + i][:], cosv[i][:])
                            nc.vector.tensor_add(qr[2 + i][:], t3[:], qr[2 + i][:])

                        # ---- attention (strict-lower-tri) + fused LN(a) ----
                        aln = [act.tile([128, T], CDT, tag=f"aln{k}",
                                        name=f"aln{k}_{hg}") for k in range(8)]
                        for j in range(4):  # t-chunks of 256
                            tj = ts(j, 256)
                            nsb = 2 * j + 2  # s-tiles 0..2j+1 are live
                            sc = [sml.tile([128, 256], CDT, tag=f"sc{i}", bufs=2,
                                           name=f"sc{i}_{hg}j{j}")
                                  for i in range(nsb)]
                            for i in range(nsb):
                                ps = P2(f"scp_{hg}j{j}i{i}")
                                for k in range(4):
                                    nc.tensor.matmul(
                                        ps[:], qr[k][:, ts(i, 128)], qr[k][:, tj],
                                        start=(k == 0), stop=(k == 3))
                                if i >= 2 * j:
                                    nc.vector.tensor_mul(sc[i][:], ps[:],
                                                         masks[i - 2 * j][:])
                                else:
                                    nc.scalar.activation(out=sc[i][:], in_=ps[:],
                                                         func=AF.Copy)
                            stp = P2(f"stp_{hg}j{j}")
                            afs = []
                            for d8 in range(8):
                                ps = PA(f"ap_{hg}j{j}d{d8}")
                                for i in range(nsb):
                                    nc.tensor.matmul(
                                        ps[:], vn_c[i][:, ts(d8, 128)], sc[i][:],
                                        start=(i == 0), stop=(i == nsb - 1))
                                af = sml.tile([128, 256], CDT, tag=f"af{d8}",
                                              bufs=1, name=f"af_{hg}j{j}d{d8}")
                                if af_act:
                                    nc.scalar.activation(out=af[:], in_=ps[:],
                                                         func=AF.Copy)
                                else:
                                    nc.vector.tensor_copy(af[:], ps[:])
                                sq = sml.tile([128, 256], CDT, tag="sq", bufs=2,
                                              name=f"sq_{hg}j{j}d{d8}")
                                nc.gpsimd.tensor_mul(sq[:], af[:], af[:])
                                nc.tensor.matmul(stp[:], ones[:], sq[:],
                                                 start=(d8 == 0), stop=(d8 == 7))
                                afs.append(af)
                            rs = sml.tile([128, 256], FP32, tag="rs", bufs=1,
                                          name=f"rs_{hg}j{j}")
                            nc.scalar.activation(out=rs[:], in_=stp[:],
                                                 func=AF.Sqrt, bias=epst[:],
                                                 scale=1.0 / D)
                            nc.vector.reciprocal(rs[:], rs[:])
                            for d8 in range(8):
                                nc.vector.tensor_mul(aln[d8][:, tj], afs[d8][:],
                                                     rs[:])

                        # ---- z = Wy^T @ aln ; y = relu(z) * x' ----
                        wyt = []
                        for k in range(8):
                            w = wst.tile([128, SD], CDT, tag="wtile",
                                         name=f"wy_{hg}k{k}")
                            nc.sync.dma_start(w[:], wy_d[h, ts(k, 128), :])
                            wyt.append(w)
                        yt = [act.tile([128, T], CDT, tag=f"y{h}_{m}",
                                       name=f"y{h}_{m}_{tg}") for m in range(4)]
                        for m in range(4):
                            for jj in range(2):
                                ps = P5(f"zps_{hg}m{m}j{jj}")
                                for k in range(8):
                                    nc.tensor.matmul(
                                        ps[:], wyt[k][:, ts(m, 128)],
                                        aln[k][:, ts(jj, 512)],
                                        start=(k == 0), stop=(k == 7))
                                rl = sml.tile([128, 512], CDT, tag="rl", bufs=2,
                                              name=f"rl_{hg}m{m}j{jj}")
                                nc.scalar.activation(out=rl[:], in_=ps[:],
                                                     func=AF.Relu)
                                nc.vector.tensor_mul(yt[m][:, ts(jj, 512)], rl[:],
                                                     xp[m][:, ts(jj, 512)])
                        ytiles[h] = yt

                    # ---- partialT[d,t] = sum_h enc_h^T @ y_h -> DRAM ----
                    for d8 in range(8):
                        ech = []
                        for kk in range(16):
                            e = est.tile([128, 128], CDT, tag="etile",
                                         name=f"e_{tg}d{d8}k{kk}")
                            nc.sync.dma_start(
                                e[:], enc_d[ts(kk, 128), ts(d8, 128)])
                            ech.append(e)
                        pss = [P5(f"ep_{tg}d{d8}j{jj}") for jj in range(2)]
                        for kk in range(16):
                            h, m = kk // 4, kk % 4
                            for jj in range(2):
                                nc.tensor.matmul(
                                    pss[jj][:], ech[kk][:],
                                    ytiles[h][m][:, ts(jj, 512)],
                                    start=(kk == 0), stop=(kk == 15))
                        for jj in range(2):
                            so = stg.tile([128, 512], FP32, tag="so", bufs=2,
                                          name=f"so_{tg}d{d8}j{jj}")
                            nc.vector.tensor_copy(so[:], pss[jj][:])
                            nc.sync.dma_start(
                                ar_in[ts(d8, 128), ts(jj, 512)], so[:])

                    # ---- AllReduce over the 4-core batch group ----
                    if collective:
                        nc.gpsimd.collective_compute(
                            "AllReduce", mybir.AluOpType.add,
                            replica_groups=[[0, 1, 2, 3], [4, 5, 6, 7]],
                            ins=[ar_in.opt()], outs=[ar_out.opt()])
                        w_src = ar_out
                    else:
                        w_src = ar_in

                    # ---- tail: u=LN(w); s=v+u; vnew=s*rsqrt(msq(s)+eps) ----
                    # vnew overwrites vT_f/vT_c/vn_c in place (old halves are
                    # dead once the s=v+u add has consumed them)
                    for jj in range(2):
                        tj = ts(jj, 512)
                        wt = [act.tile([128, 512], FP32, tag=f"wt{k}",
                                       name=f"wt{k}_{tg}j{jj}") for k in range(8)]
                        mwp = P5(f"mwp_{tg}j{jj}")
                        msp = P5(f"msp_{tg}j{jj}")
                        for k in range(8):
                            nc.sync.dma_start(wt[k][:], w_src[ts(k, 128), tj])
                            wb = sml.tile([128, 512], CDT, tag="rl", bufs=2,
                                          name=f"wb_{tg}j{jj}k{k}")
                            nc.scalar.activation(out=wb[:], in_=wt[k][:],
                                                 func=AF.Copy)
                            sq = sml.tile([128, 512], CDT, tag="rl", bufs=2,
                                          name=f"wsq_{tg}j{jj}k{k}")
                            nc.scalar.activation(out=sq[:], in_=wt[k][:],
                                                 func=AF.Square)
                            nc.tensor.matmul(mwp[:], ones[:], wb[:],
                                             start=(k == 0), stop=(k == 7))
                            nc.tensor.matmul(msp[:], ones[:], sq[:],
                                             start=(k == 0), stop=(k == 7))
                        mwn = sml.tile([128, 512], FP32, tag="mwn", bufs=1,
                                       name=f"mwn_{tg}j{jj}")
                        nc.scalar.activation(out=mwn[:], in_=mwp[:], func=AF.Copy,
                                             scale=1.0 / D)
                        m2 = sml.tile([128, 512], FP32, tag="m2", bufs=1,
                                      name=f"m2_{tg}j{jj}")
                        nc.vector.tensor_mul(m2[:], mwn[:], mwn[:])
                        rsw = sml.tile([128, 512], FP32, tag="rsw",
                                       name=f"rsw_{tg}j{jj}")
                        nc.scalar.activation(out=rsw[:], in_=msp[:], func=AF.Copy,
                                             scale=1.0 / D)
                        nc.vector.tensor_sub(rsw[:], rsw[:], m2[:])
                        nc.scalar.activation(out=rsw[:], in_=rsw[:], func=AF.Sqrt,
                                             bias=epst[:], scale=1.0)
                        nc.vector.reciprocal(rsw[:], rsw[:])
                        ssp = P5(f"ssp_{tg}j{jj}")
                        for k in range(8):
                            nc.vector.tensor_sub(wt[k][:], wt[k][:], mwn[:])
                            nc.vector.tensor_mul(wt[k][:], wt[k][:], rsw[:])
                            nc.gpsimd.tensor_add(wt[k][:], wt[k][:],
                                                 vT_f[k][:, tj])
                            sq = sml.tile([128, 512], CDT, tag="rl", bufs=2,
                                          name=f"ssq_{tg}j{jj}k{k}")
                            nc.scalar.activation(out=sq[:], in_=wt[k][:],
                                                 func=AF.Square)
                            nc.tensor.matmul(ssp[:], ones[:], sq[:],
                                             start=(k == 0), stop=(k == 7))
                        rss = sml.tile([128, 512], FP32, tag="rsw",
                                       name=f"rss_{tg}j{jj}")
                        nc.scalar.activation(out=rss[:], in_=ssp[:], func=AF.Sqrt,
                                             bias=epst[:], scale=1.0 / D)
                        nc.vector.reciprocal(rss[:], rss[:])
                        for k in range(8):
                            nc.vector.tensor_mul(vT_f[k][:, tj], wt[k][:], rss[:])
                            nc.scalar.activation(out=vT_c[k][:, tj],
                                                 in_=vT_f[k][:, tj], func=AF.Copy)
                    # transpose vnew -> natural (bf16) via DMA xbar
                    for a in range(8):
                        for bb in range(8):
                            nc.sync.dma_start_transpose(
                                vn_c[bb][:, ts(a, 128)], vT_c[a][:, ts(bb, 128)])

            # ---- readout: logits = v^T @ readout_shard ----
            if do_readout:
                for nn_ in range(NVC):
                    rot = []
                    for k in range(8):
                        rtile = wst.tile([128, VCH], CDT, tag="rtile", bufs=8,
                                         name=f"ro_n{nn_}k{k}")
                        nc.sync.dma_start(
                            rtile[:], ro_d[ts(k, 128), ts(nn_, VCH)])
                        rot.append(rtile)
                    for m in range(8):
                        ps = P5(f"rps_n{nn_}m{m}")
                        for k in range(8):
                            nc.tensor.matmul(ps[:, 0:VCH],
                                             vT_c[k][:, ts(m, 128)], rot[k][:],
                                             start=(k == 0), stop=(k == 7))
                        ot = stg.tile([128, VCH], FP32, tag="so", bufs=2,
                                      name=f"ot_n{nn_}m{m}")
                        if m % 2 == 0:
                            nc.vector.tensor_copy(ot[:], ps[:, 0:VCH])
                        else:
                            nc.scalar.activation(out=ot[:], in_=ps[:, 0:VCH],
                                                 func=AF.Copy)
                        nc.sync.dma_start(
                            out_d[ts(m, 128), ts(nn_, VCH)], ot[:])
    nc.compile()
    return nc


def host_prep(inputs):
    idx = np.asarray(inputs["idx"])
    wte = np.asarray(inputs["wte"], np.float32)
    enc = np.asarray(inputs["encoder"], np.float32)
    dx = np.asarray(inputs["decoder_x"], np.float32)
    dy = np.asarray(inputs["decoder_y"], np.float32)
    ro = np.asarray(inputs["readout"], np.float32)
    bf = ml_dtypes.bfloat16

    perm = np.concatenate([np.arange(0, SD, 2), np.arange(1, SD, 2)])
    Wx = np.ascontiguousarray(dx[:, :, perm])                       # [H, D, SD]
    Wy = np.ascontiguousarray(dy[:, :, perm])
    encp = np.ascontiguousarray(enc.reshape(H, SD, D)[:, perm, :])  # [H, SD, D]

    g = wte[idx]                                                    # [B, T, D]
    m = g.mean(-1, keepdims=True)
    var = ((g - m) ** 2).mean(-1, keepdims=True)
    v0 = (g - m) / np.sqrt(var + EPS)

    inv_freq = 1.0 / (10000.0 ** (np.arange(0, SD, 2, dtype=np.float32) / SD))
    freqs = np.arange(T, dtype=np.float32)[None, :] * inv_freq[:, None]
    cosT = np.cos(freqs).astype(np.float32)                         # [SD/2, T]
    sinT = np.sin(freqs).astype(np.float32)

    ss, tt = np.mgrid[0:128, 0:256]
    msk = np.stack([(tt > ss), (tt > ss + 128)]).astype(np.float32)

    in_maps = []
    for c in range(NCORES):
        b, hs = c // 4, c % 4
        hsl = slice(4 * hs, 4 * hs + 4)
        v0T = np.ascontiguousarray(v0[b].T)
        in_maps.append({
            "v0t_f": v0T,
            "v0t_c": v0T.astype(bf),
            "v0n_c": np.ascontiguousarray(v0[b]).astype(bf),
            "wx": Wx[hsl].astype(bf),
            "wy": Wy[hsl].astype(bf),
            "enc": np.ascontiguousarray(encp[hsl].reshape(NHC * SD, D)).astype(bf),
            "ro": np.ascontiguousarray(ro[:, VSH * hs: VSH * (hs + 1)]).astype(bf),
            "cos": cosT.astype(bf),
            "sin": sinT.astype(bf),
            "msk": msk.astype(bf),
        })
    return in_maps


def make_runner(nc, n_cores=NCORES):
    import jax
    from jax.sharding import Mesh, PartitionSpec
    from jax.experimental.shard_map import shard_map

    bass2jax.install_neuronx_cc_hook()
    partition_name = nc.partition_id_tensor.name if nc.partition_id_tensor else None
    in_names, out_names, out_avals, zero_shapes = [], [], [], []
    for alloc in nc.m.functions[0].allocations:
        if not isinstance(alloc, mybir.MemoryLocationSet):
            continue
        name = alloc.memorylocations[0].name
        if alloc.kind == "ExternalInput":
            if name != partition_name:
                in_names.append(name)
        elif alloc.kind == "ExternalOutput":
            shape = tuple(alloc.tensor_shape)
            dtype = mybir.dt.np(alloc.dtype)
            out_names.append(name)
            out_avals.append(jax.core.ShapedArray(shape, dtype))
            zero_shapes.append((shape, dtype))
    n_params, n_outs = len(in_names), len(out_avals)
    all_in = list(in_names) + list(out_names)
    if partition_name is not None:
        all_in.append(partition_name)

    def _body(*args):
        operands = list(args)
        if partition_name is not None:
            operands.append(bass2jax.partition_id_tensor())
        return tuple(bass2jax._bass_exec_p.bind(
            *operands, out_avals=tuple(out_avals), in_names=tuple(all_in),
            out_names=tuple(out_names), lowering_input_output_aliases=(),
            sim_require_finite=True, sim_require_nnan=True, nc=nc))

    devices = jax.devices()[:n_cores]
    mesh = Mesh(np.asarray(devices), ("core",))
    f = jax.jit(
        shard_map(_body, mesh=mesh,
                  in_specs=(PartitionSpec("core"),) * (n_params + n_outs),
                  out_specs=(PartitionSpec("core"),) * n_outs, check_rep=False),
        keep_unused=True)

    def prep(in_maps):
        concat = [np.concatenate([np.asarray(in_maps[c][k])
                                  for c in range(n_cores)], axis=0)
                  for k in in_names]
        zeros = [np.zeros((n_cores * s[0], *s[1:]), dt) for (s, dt) in zero_shapes]
        return [jax.device_put(x) for x in concat + zeros]

    def run(dev_args):
        outs = f(*dev_args)
        jax.block_until_ready(outs)
        return outs

    def split(outs):
        return [{name: np.asarray(outs[i]).reshape(n_cores, *out_avals[i].shape)[c]
                 for i, name in enumerate(out_names)} for c in range(n_cores)]

    return run, prep, split


def kernel(**inputs) -> np.ndarray:
    if "prog" not in _CACHE:
        nc = build_program()
        _CACHE["prog"] = nc
        _CACHE["runner"] = make_runner(nc)
    run, prep, split = _CACHE["runner"]
    in_maps = host_prep(inputs)
    args = prep(in_maps)
    res = split(run(args))
    out = np.zeros((B, T, VOCAB), np.float32)
    for c in range(NCORES):
        b, hs = c // 4, c % 4
        out[b, :, VSH * hs: VSH * (hs + 1)] = res[c]["logits"]
    return out
